# revision 25
# baseline (speedup 1.0000x reference)
"""Trainium2 Bass kernel for nn_DechunkingLayer (ragged_sequence).

Reference semantics (per batch row):
    idx = clip(exclusive_cumsum(b), 0, NC - 1)          # [T]
    up[t]  = z[idx[t]]                                  # gather rows
    out[t] = p[t] * up[t] + (1 - p[t]) * up[t-1]        # EMA blend
    out[0] = up[0]

Sharding: pure data parallel over batch B=8 across the 8 NeuronCores.

Per-core plan (v7 — v6 structure + prologue/ramp overhaul + engine
rebalance).  Kept from v6 (see kernel_v6_backup.py): dedup window
gathers with OOB-skip, one-hot fp32 PE expansion (bitwise exact),
DVE stream-shuffle roll + BR boundary-fix, exact fp32 blend order.

New in v7:
  * PE keep-alive: warm-up matmuls start at t~0.5us (inputs are DVE
    memsets, no other deps) and are interleaved through the prologue so
    the HAM clock ramps to full speed before the scan and never drops.
    (v6 ran its first ~8 tiles at half clock, costing ~15-20us.)
  * Prologue critical path: b load -> transpose -> cumsum -> gidx is
    scheduled first; everything else (BR/bidx chain, p/q transposes,
    rank scratch roundtrip) comes after the window gathers are already
    dispatched on the gpsimd queue.
  * First-lap windows all use OOB-marked gathers (ring bufs 1..7 are
    memset during the ramp); saves the clamped full-128-row fetches.
  * W matrices for tiles 0..7 are built via PE row-broadcasts (no DRAM
    scratch roundtrip on the critical path); tiles 8+ come from the
    fp16 replicate-gather tables, with one batched is_equal producing
    8 tiles' W at a time ([128, 1024] compare).
  * Blend rebalance: ACT computes t1 = p*up AND t2 = q*roll (two
    activation streams); the final add is split DVE [0:768] /
    GpSimd [768:1024] (gpsimd TensorTensor is ~2.6x slower per elem).
    DVE per tile: shuffle + 3/4 add ~ 2.5us, under the PE pace
    (4 fp32 passes ~ 2.9us/tile).  fl(t1 + t2) with t1 = fl(p*up),
    t2 = fl(q*roll) is bitwise identical to the reference.
  * fix-DMA + out stores dispatch from the sync queue (scalar/ACT is
    busy with t1/t2).
"""

import numpy as np

import concourse.bacc as bacc
import concourse.bass as bass
import concourse.mybir as mybir
import concourse.tile as tile
from concourse.bass import IndirectOffsetOnAxis
from concourse.bass_utils import run_bass_kernel_spmd
from concourse.masks import make_identity, make_upper_triangular

# Problem shape (hardcoded per harness contract).
B = 8          # batch rows == number of cores
T = 4096       # timesteps per row
NCH = 2048     # number of chunks (z rows)
D = 1024       # d_model
P = 128        # SBUF partitions
NT = T // P    # 32 tiles per core
NCOL = T // P  # 32 columns in the W layout
DH = D // 2    # matmul free-dim max for fp32 is 512

F32 = mybir.dt.float32
F16 = mybir.dt.float16
I32 = mybir.dt.int32

BIG = float(1 << 20)   # OOB marker offset for skipped gather rows
ZG_BUFS = 8            # gather window ring (how far gathers run ahead)
NCHUNK = 4             # rank-table replication chunks (8 tiles each)
TPC = NT // NCHUNK     # tiles per chunk (8)
NBCAST = TPC           # tiles whose W comes from the PE-broadcast path
DSPL = D               # add split: DVE [0:DSPL], GpSimd [DSPL:D]
# stream_shuffle: out[32a+r] = in[32a+mask[r]]  ->  shift down by one
SHIFT_MASK = [0] + list(range(0, 31))


def build_bass() -> bass.Bass:
    nc = bacc.Bacc()

    z = nc.dram_tensor("z", [NCH, D], F32, kind="ExternalInput")
    p = nc.dram_tensor("p", [T], F32, kind="ExternalInput")
    b = nc.dram_tensor("b", [T], I32, kind="ExternalInput")
    out = nc.dram_tensor("out", [T, D], F32, kind="ExternalOutput")
    # DRAM scratch for the fp16 rank-table broadcast roundtrip
    scratch = nc.dram_tensor("scratch", [NCOL, P], F16, kind="Internal")

    with tile.TileContext(nc) as tc:
        with (
            tc.tile_pool(name="setup", bufs=1) as sp,
            tc.tile_pool(name="psmall", bufs=2, space="PSUM") as pps,
            tc.tile_pool(name="pmm", bufs=3, space="PSUM") as pmm,
            tc.tile_pool(name="wpool", bufs=2) as wp,
            tc.tile_pool(name="roll", bufs=4) as rp,
            tc.tile_pool(name="tmul", bufs=4) as tp,
            tc.tile_pool(name="opool", bufs=4) as op,
            tc.tile_pool(name="zg", bufs=ZG_BUFS) as zp,
        ):
            # ---- input loads + tile-0 window prefetch (no dependencies) ----
            b2d = b[:].rearrange("(j c) -> j c", c=P)          # [32, 128] DRAM
            p2d = p[:].rearrange("(j c) -> j c", c=P)

            b_nat_i = sp.tile([NCOL, P], I32)
            nc.sync.dma_start(out=b_nat_i[:], in_=b2d)
            # tile-0 window prefetch: z[0:128] interleaved to match j(q)
            # (j(q) = 2q for q<64, 2q-127 for q>=64), dependency-free.
            zg0 = zp.tile([P, D], F32, tag="zg")
            nc.sync.dma_start(out=zg0[0:64, :], in_=z[0 : P - 1 : 2, :])
            nc.sync.dma_start(out=zg0[64:P, :], in_=z[1:P:2, :])
            p_nat = sp.tile([NCOL, P], F32)
            nc.sync.dma_start(out=p_nat[:], in_=p2d)

            # b_shifted[t] = b[t-1] (0 at t=0)
            bp_nat_i = sp.tile([NCOL, P], I32)
            nc.vector.memset(bp_nat_i[0:1, 0:1], 0)
            nc.sync.dma_start(out=bp_nat_i[:, 1:P], in_=b2d[:, 0 : P - 1])
            nc.sync.dma_start(
                out=bp_nat_i[1:NCOL, 0:1], in_=b2d[0 : NCOL - 1, P - 1 : P]
            )

            # ---- PE keep-alive: warm matmuls with memset-only inputs -------
            ones_pp = sp.tile([P, P], F32)
            nc.vector.memset(ones_pp[:], 1.0)
            warm_src = sp.tile([P, DH], F32)
            nc.vector.memset(warm_src[:], 1.0)

            def warm_mm(n):
                for _ in range(n):
                    wps = pmm.tile([P, D], F32, space="PSUM", tag="mm")
                    nc.tensor.matmul(out=wps[:, 0:DH], lhsT=ones_pp[:],
                                     rhs=warm_src[:], start=True, stop=True,
                                     skip_group_check=True)

            warm_mm(1)

            # ---- gpsimd constants (before the gather stream on its FIFO) ---
            tri_g = sp.tile([P, P], F32)     # tri[k, i] = 1 iff i > k
            make_upper_triangular(nc, tri_g[:], val=1.0, diag=False)
            ident_g = sp.tile([NCOL, NCOL], F32)
            make_identity(nc, ident_g[:])
            ident128_g = sp.tile([P, P], F32)
            make_identity(nc, ident128_g[:])
            tri32_g = sp.tile([NCOL, NCOL], F32)
            make_upper_triangular(nc, tri32_g[:], val=1.0, diag=False)
            iotap_i = sp.tile([P, 1], I32)   # iotap[q] = q
            nc.gpsimd.iota(iotap_i[:], pattern=[[0, 1]], base=0,
                           channel_multiplier=1)
            cidx_i = sp.tile([P, NCHUNK], I32)   # cidx[q, c] = c
            nc.gpsimd.iota(cidx_i[:], pattern=[[1, NCHUNK]], base=0,
                           channel_multiplier=0)
            jrow_i = sp.tile([P, NCOL], I32)     # jrow[q, j] = j
            nc.gpsimd.iota(jrow_i[:], pattern=[[1, NCOL]], base=0,
                           channel_multiplier=0)
            # wide ones for the iota2 broadcast tables (gpsimd: DVE is busy)
            ones_w16 = sp.tile([P, TPC * P], F16)
            nc.gpsimd.memset(ones_w16[:], 1.0)
            ones_w32 = sp.tile([P, TPC * P], F32)
            nc.gpsimd.memset(ones_w32[:], 1.0)

            # ---- DVE: scan-critical copies first ---------------------------
            ident = sp.tile([NCOL, NCOL], F32)
            nc.vector.tensor_copy(out=ident[:], in_=ident_g[:])
            b_nat = sp.tile([NCOL, P], F32)
            nc.vector.tensor_copy(out=b_nat[:], in_=b_nat_i[:])
            bp_nat = sp.tile([NCOL, P], F32)
            nc.vector.tensor_copy(out=bp_nat[:], in_=bp_nat_i[:])
            tri = sp.tile([P, P], F32)
            nc.vector.tensor_copy(out=tri[:], in_=tri_g[:])
            tri32 = sp.tile([NCOL, NCOL], F32)
            nc.vector.tensor_copy(out=tri32[:], in_=tri32_g[:])
            ident128 = sp.tile([P, P], F32)
            nc.vector.tensor_copy(out=ident128[:], in_=ident128_g[:])
            iotap_f = sp.tile([P, 1], F32)
            nc.vector.tensor_copy(out=iotap_f[:], in_=iotap_i[:])

            ones_row = sp.tile([1, P], F32)
            nc.vector.memset(ones_row[:], 1.0)
            ones_col = sp.tile([P, 1], F32)
            nc.vector.memset(ones_col[:], 1.0)

            # interleaved window offsets: j(q) = 2q (q<64), 2q-127 (q>=64)
            q2 = sp.tile([P, 1], F32)
            nc.vector.tensor_scalar_mul(out=q2[:], in0=iotap_f[:], scalar1=2.0)
            qm = sp.tile([P, 1], F32)
            nc.vector.tensor_scalar_min(out=qm[:], in0=iotap_f[:], scalar1=64.0)
            qhi = sp.tile([P, 1], F32)     # 1.0 iff q >= 64
            nc.vector.tensor_single_scalar(out=qhi[:], in_=qm[:], scalar=64.0,
                                           op=mybir.AluOpType.is_equal)
            iota2_col = sp.tile([P, 1], F32)   # j(q)
            nc.vector.scalar_tensor_tensor(
                out=iota2_col[:], in0=qhi[:], scalar=-127.0, in1=q2[:],
                op0=mybir.AluOpType.mult, op1=mybir.AluOpType.add,
            )
            # iota2 tiled compare tables (ACT queue, idle this early)
            iota2_t16 = sp.tile([P, TPC * P], F16)
            nc.scalar.mul(out=iota2_t16[:], in_=ones_w16[:], mul=iota2_col[:])
            iota2_t32 = sp.tile([P, TPC * P], F32)
            nc.scalar.mul(out=iota2_t32[:], in_=ones_w32[:], mul=iota2_col[:])
            ones_row_bf = sp.tile([1, P], mybir.dt.bfloat16)
            nc.vector.memset(ones_row_bf[:], 1.0)
            # ---- scan: transposes to W layout [128, 32], cumsum ------------
            bw_ps = pps.tile([P, NCOL], F32, space="PSUM", tag="small_ps")
            nc.tensor.transpose(out=bw_ps[:], in_=b_nat[:], identity=ident[:])
            b_w = sp.tile([P, NCOL], F32)
            nc.vector.tensor_copy(out=b_w[:], in_=bw_ps[:])

            bpw_ps = pps.tile([P, NCOL], F32, space="PSUM", tag="small_ps")
            nc.tensor.transpose(out=bpw_ps[:], in_=bp_nat[:], identity=ident[:])
            bp_w = sp.tile([P, NCOL], F32)
            nc.vector.tensor_copy(out=bp_w[:], in_=bpw_ps[:])

            totc_ps = pps.tile([NCOL, 1], F32, space="PSUM", tag="small_ps")
            nc.tensor.matmul(out=totc_ps[:], lhsT=b_w[:], rhs=ones_col[:],
                             start=True, stop=True)
            tot_col = sp.tile([NCOL, 1], F32)
            nc.vector.tensor_copy(out=tot_col[:], in_=totc_ps[:])
            cofs_ps = pps.tile([1, NCOL], F32, space="PSUM", tag="small_ps")
            nc.tensor.matmul(out=cofs_ps[:], lhsT=tot_col[:], rhs=tri32[:],
                             start=True, stop=True)
            colofs = sp.tile([1, NCOL], F32)
            nc.vector.tensor_copy(out=colofs[:], in_=cofs_ps[:])

            s_ps = pps.tile([P, NCOL], F32, space="PSUM", tag="small_ps")
            nc.tensor.matmul(out=s_ps[:], lhsT=tri[:], rhs=b_w[:],
                             start=True, stop=False)
            nc.tensor.matmul(out=s_ps[:], lhsT=ones_row[:], rhs=colofs[:],
                             start=False, stop=True)

            # idx = min(s, NCH-1); idxp = min(s - b_shifted, NCH-1).
            # Both consume s_ps HERE, before the pps ring recycles its bank.
            idx_f = sp.tile([P, NCOL], F32)
            nc.vector.tensor_scalar_min(out=idx_f[:], in0=s_ps[:],
                                        scalar1=float(NCH - 1))
            sprev_f = sp.tile([P, NCOL], F32)
            nc.vector.tensor_sub(out=sprev_f[:], in0=s_ps[:], in1=bp_w[:])
            idxp_f = sp.tile([P, NCOL], F32)
            nc.vector.tensor_scalar_min(out=idxp_f[:], in0=sprev_f[:],
                                        scalar1=float(NCH - 1))
            # last_row[k] = idx[128k+127] via tiny DMA (scalar queue, idle)
            last_row = sp.tile([1, NCOL], F32)
            nc.scalar.dma_start(out=last_row[:], in_=idx_f[P - 1 : P, :])

            basesb_ps = pps.tile([P, NCOL], F32, space="PSUM", tag="small_ps")
            nc.tensor.matmul(out=basesb_ps[:], lhsT=ones_row[:],
                             rhs=idx_f[0:1, :], start=True, stop=True)
            lastb_ps = pps.tile([P, NCOL], F32, space="PSUM", tag="small_ps")
            nc.tensor.matmul(out=lastb_ps[:], lhsT=ones_row[:],
                             rhs=last_row[:], start=True, stop=True)

            # rank table [128, 32] (one-hot positions within the window)
            rank1 = sp.tile([P, NCOL], F32)
            nc.vector.tensor_sub(out=rank1[:], in0=idx_f[:], in1=basesb_ps[:])
            # rank -> fp16, transposed; W path critical
            r1t_ps = pps.tile([NCOL, P], F32, space="PSUM", tag="small_ps")
            nc.tensor.transpose(out=r1t_ps[:], in_=rank1[:],
                                identity=ident128[:])
            rank1t16 = sp.tile([NCOL, P], F16)
            nc.vector.tensor_copy(out=rank1t16[:], in_=r1t_ps[:])
            # group-0 rank rows flattened [8,128] -> [1,1024] in one DMA
            # (partition-major element order matches k*128+t)
            g0row16 = sp.tile([1, TPC * P], F16)
            nc.scalar.dma_start(out=g0row16[0:1, :], in_=rank1t16[0:TPC, :])

            # gather indices: gidx[q, k] = bases[k] + j(q); OOB-marked past
            # last_k so the indirect DMA skips those rows entirely.  The
            # first-lap windows (k=1..4) use CLAMPED indices instead: every
            # slot is fetched, so the ring bufs need no memset.
            graw = sp.tile([P, NCOL], F32)
            nc.vector.tensor_scalar_add(out=graw[:], in0=basesb_ps[:],
                                        scalar1=iota2_col[:])
            gidxc_f = sp.tile([P, NCOL], F32)
            nc.vector.tensor_scalar_min(out=gidxc_f[:], in0=graw[:],
                                        scalar1=float(NCH - 1))
            gidxc_i = sp.tile([P, NCOL], I32)
            nc.vector.tensor_copy(out=gidxc_i[:], in_=gidxc_f[:])
            delta = sp.tile([P, NCOL], F32)
            nc.vector.tensor_sub(out=delta[:], in0=lastb_ps[:], in1=graw[:])
            dmin = sp.tile([P, NCOL], F32)
            nc.vector.tensor_scalar_min(out=dmin[:], in0=delta[:], scalar1=0.0)
            vld = sp.tile([P, NCOL], F32)
            nc.vector.tensor_single_scalar(out=vld[:], in_=dmin[:], scalar=0.0,
                                           op=mybir.AluOpType.is_equal)
            graw_big = sp.tile([P, NCOL], F32)
            nc.vector.tensor_scalar_add(out=graw_big[:], in0=graw[:],
                                        scalar1=BIG)
            gidx_f = sp.tile([P, NCOL], F32)
            nc.vector.scalar_tensor_tensor(
                out=gidx_f[:], in0=vld[:], scalar=-BIG, in1=graw_big[:],
                op0=mybir.AluOpType.mult, op1=mybir.AluOpType.add,
            )
            gidx_i = sp.tile([P, NCOL], I32)
            nc.vector.tensor_copy(out=gidx_i[:], in_=gidx_f[:])

            zg_ring = {}
            for i in range(1, ZG_BUFS):
                zg_ring[i] = zp.tile([P, D], F32, tag="zg", name=f"zgr{i}")
            zgs = {0: zg0}

            def window_gather(k):
                if k >= NT:
                    return
                zg = (zg_ring[k] if 0 < k < ZG_BUFS
                      else zp.tile([P, D], F32, tag="zg", name=f"zgw{k}"))
                if k < 5:
                    nc.gpsimd.indirect_dma_start(
                        out=zg[:], out_offset=None, in_=z[:],
                        in_offset=IndirectOffsetOnAxis(
                            ap=gidxc_i[:, k : k + 1], axis=0),
                    )
                else:
                    nc.gpsimd.indirect_dma_start(
                        out=zg[:], out_offset=None, in_=z[:],
                        in_offset=IndirectOffsetOnAxis(
                            ap=gidx_i[:, k : k + 1], axis=0),
                        bounds_check=NCH - 1, oob_is_err=False,
                    )
                zgs[k] = zg

            # ---- first gather wave, then the chunk-table roundtrip ---------
            for k in range(1, 5):
                window_gather(k)
            nc.gpsimd.dma_start(out=scratch[:], in_=rank1t16[:])
            scr_flat = scratch[:].rearrange("(a k) c -> a (k c)", a=NCHUNK)

            rbs = [None] + [sp.tile([P, TPC * P], F16, name=f"rb16_{c}")
                            for c in range(1, NCHUNK)]

            def chunk_gather(c):
                nc.gpsimd.indirect_dma_start(
                    out=rbs[c][:], out_offset=None, in_=scr_flat,
                    in_offset=IndirectOffsetOnAxis(ap=cidx_i[:, c : c + 1],
                                                   axis=0),
                )

            chunk_gather(1)

            # group-0 W: bf16 PE broadcast (exact: rank < 128) + compare
            g0row_bf = sp.tile([1, TPC * P], mybir.dt.bfloat16)
            nc.vector.tensor_copy(out=g0row_bf[:], in_=g0row16[:])
            rbig = pmm.tile([P, D], F32, space="PSUM", tag="mm", name="rbig")
            for h in range(2):
                sl = slice(h * DH, (h + 1) * DH)
                nc.tensor.matmul(out=rbig[:, sl], lhsT=ones_row_bf[:],
                                 rhs=g0row_bf[0:1, sl], start=True, stop=True,
                                 skip_group_check=True)
            w8s = {}
            w8_0 = wp.tile([P, TPC * P], F32, tag="w8", name="w8_0")
            nc.vector.tensor_tensor(out=w8_0[:], in0=iota2_t32[:],
                                    in1=rbig[:],
                                    op=mybir.AluOpType.is_equal)
            w8s[0] = w8_0

            def build_w8(c):
                w8 = wp.tile([P, TPC * P], F32, tag="w8", name=f"w8_{c}")
                nc.vector.tensor_tensor(out=w8[:], in0=iota2_t16[:],
                                        in1=rbs[c][:],
                                        op=mybir.AluOpType.is_equal)
                w8s[c] = w8

            # ---- aux: p/q transposes, diag extracts, BR/upB gathers --------
            pw_ps = pps.tile([P, NCOL], F32, space="PSUM", tag="small_ps")
            nc.tensor.transpose(out=pw_ps[:], in_=p_nat[:], identity=ident[:])
            p_w = sp.tile([P, NCOL], F32)
            nc.vector.tensor_copy(out=p_w[:], in_=pw_ps[:])
            # out[0] = up[0] exactly: p[0] = 1 so the blend is 1*up + 0*rolled
            nc.vector.memset(p_w[0:1, 0:1], 1.0)
            q_w = sp.tile([P, NCOL], F32)  # q = 1 - p
            nc.scalar.activation(
                out=q_w[:], in_=p_w[:],
                func=mybir.ActivationFunctionType.Copy, bias=1.0, scale=-1.0,
            )

            # diagonal extraction helpers: diag(X)[32a + k] = X[32a, k]
            jrow_f = sp.tile([P, NCOL], F32)
            nc.vector.tensor_copy(out=jrow_f[:], in_=jrow_i[:])
            qa = sp.tile([P, 1], F32)      # floor(q / 32) via 3 thresholds
            nc.vector.memset(qa[:], 0.0)
            for thr in (32.0, 64.0, 96.0):
                qt = sp.tile([P, 1], F32, name=f"qt{int(thr)}")
                nc.vector.tensor_scalar_min(out=qt[:], in0=iotap_f[:],
                                            scalar1=thr)
                qi = sp.tile([P, 1], F32, name=f"qi{int(thr)}")
                nc.vector.tensor_single_scalar(out=qi[:], in_=qt[:], scalar=thr,
                                               op=mybir.AluOpType.is_equal)
                nc.vector.tensor_add(out=qa[:], in0=qa[:], in1=qi[:])
            qmod = sp.tile([P, 1], F32)
            nc.vector.scalar_tensor_tensor(
                out=qmod[:], in0=qa[:], scalar=-32.0, in1=iotap_f[:],
                op0=mybir.AluOpType.mult, op1=mybir.AluOpType.add,
            )
            eye = sp.tile([P, NCOL], F32)
            nc.vector.tensor_scalar(out=eye[:], in0=jrow_f[:], scalar1=qmod[:],
                                    scalar2=None,
                                    op0=mybir.AluOpType.is_equal)

            def diag_extract(x, name):
                bqt = sp.tile([P, NCOL], F32, name=f"dq_{name}")
                nc.vector.stream_shuffle(out=bqt[:], in_=x[:], mask=[0] * 32)
                prod = sp.tile([P, NCOL], F32, name=f"dp_{name}")
                nc.vector.tensor_mul(out=prod[:], in0=bqt[:], in1=eye[:])
                dcol = sp.tile([P, 1], F32, name=f"dc_{name}")
                nc.vector.tensor_reduce(out=dcol[:], in_=prod[:],
                                        axis=mybir.AxisListType.X,
                                        op=mybir.AluOpType.add)
                return dcol

            # boundary-row blend inputs: for j = 32a + k (tile k, row 32a):
            #   BR[j]  = z[idxp[128k + 32a]]   (rolled value)
            #   upB[j] = z[idx[128k + 32a]]    (up value)
            #   pd[j]  = p[128k + 32a], qd = 1 - pd
            bidx_f = diag_extract(idxp_f, "b")
            bidx_i = sp.tile([P, 1], I32)
            nc.vector.tensor_copy(out=bidx_i[:], in_=bidx_f[:])
            uidx_f = diag_extract(idx_f, "u")
            uidx_i = sp.tile([P, 1], I32)
            nc.vector.tensor_copy(out=uidx_i[:], in_=uidx_f[:])
            pd = diag_extract(p_w, "p")
            qd = sp.tile([P, 1], F32)
            nc.scalar.activation(
                out=qd[:], in_=pd[:],
                func=mybir.ActivationFunctionType.Copy, bias=1.0, scale=-1.0,
            )

            # ring bufs 5..7 need memsets (their gathers are OOB-marked)
            for i in range(5, ZG_BUFS):
                nc.vector.memset(zg_ring[i][:], 0.0)

            window_gather(5)
            br = sp.tile([P, D], F32)
            nc.gpsimd.indirect_dma_start(
                out=br[:], out_offset=None, in_=z[:],
                in_offset=IndirectOffsetOnAxis(ap=bidx_i[:, 0:1], axis=0),
            )
            upb = sp.tile([P, D], F32)
            nc.gpsimd.indirect_dma_start(
                out=upb[:], out_offset=None, in_=z[:],
                in_offset=IndirectOffsetOnAxis(ap=uidx_i[:, 0:1], axis=0),
            )
            window_gather(6)
            window_gather(7)

            # ---- main loop -------------------------------------------------
            # Per tile k this iteration emits: matmul(k), shuffle(k), t1(k)
            # (frees up_ps), then the k-1 tail: t2, split add, boundary-row
            # fix from oB (WAW on o, off every compute chain), store.
            pend = {}   # j -> (roll, t1)
            ob_box = []

            def emit_tail(j):
                roll_j, t1_j = pend.pop(j)
                t2 = tp.tile([P, D], F32, tag="t2", name=f"t2_{j}")
                nc.scalar.mul(out=t2[:], in_=roll_j[:], mul=q_w[:, j : j + 1])
                o = op.tile([P, D], F32, tag="o", name=f"o{j}")
                nc.vector.tensor_add(out=o[:], in0=t1_j[:], in1=t2[:])
                # quadrant-boundary rows {0,32,64,96} come from oB
                nc.sync.dma_start(out=o[0:P:NCOL, :],
                                  in_=ob_box[0][j : P : NCOL, :])
                nc.sync.dma_start(out=out[j * P : (j + 1) * P, :], in_=o[:])

            for k in range(NT):
                window_gather(k + ZG_BUFS)       # keep the ring k+8 ahead
                if k == 2:
                    chunk_gather(2)
                if k == 10:
                    chunk_gather(3)
                # batched W for chunk c ready ~6 tiles ahead of first use
                if k == 2:
                    build_w8(1)
                if k == 10:
                    build_w8(2)
                if k == 18:
                    build_w8(3)

                c = k // TPC
                o8 = (k % TPC) * P
                wap = w8s[c][:, o8 : o8 + P]
                zg = zgs.pop(k)

                up_ps = pmm.tile([P, D], F32, space="PSUM", tag="mm",
                                 name=f"up{k}")
                for h in range(2):
                    sl = slice(h * DH, (h + 1) * DH)
                    nc.tensor.matmul(out=up_ps[:, sl], lhsT=wap,
                                     rhs=zg[:, sl], start=True, stop=True,
                                     skip_group_check=True)

                roll = rp.tile([P, D], F32, tag="roll", name=f"roll{k}")
                nc.vector.stream_shuffle(out=roll[:], in_=up_ps[:],
                                         mask=SHIFT_MASK)
                t1 = tp.tile([P, D], F32, tag="t1", name=f"t1_{k}")
                nc.scalar.mul(out=t1[:], in_=up_ps[:], mul=p_w[:, k : k + 1])
                pend[k] = (roll, t1)

                if k == 1:
                    # oB = fl(pd*upB) + fl(qd*BR): the 128 boundary out rows.
                    # Emitted here so its gather latency never blocks the
                    # main-loop queues; first use is emit_tail(0) at k=2.
                    t1b = sp.tile([P, D], F32)
                    nc.scalar.mul(out=t1b[:], in_=upb[:], mul=pd[:])
                    t2b = sp.tile([P, D], F32)
                    nc.scalar.mul(out=t2b[:], in_=br[:], mul=qd[:])
                    ob = sp.tile([P, D], F32)
                    nc.vector.tensor_add(out=ob[:], in0=t1b[:], in1=t2b[:])
                    ob_box.append(ob)

                if k >= 2:
                    emit_tail(k - 2)

            emit_tail(NT - 2)
            emit_tail(NT - 1)

    nc.finalize()
    return nc


_NC_CACHE = None


def _get_nc() -> bass.Bass:
    global _NC_CACHE
    if _NC_CACHE is None:
        _NC_CACHE = build_bass()
    return _NC_CACHE


def make_in_maps(z: np.ndarray, p: np.ndarray, b: np.ndarray) -> list[dict]:
    return [
        {
            "z": np.ascontiguousarray(z[i], dtype=np.float32),
            "p": np.ascontiguousarray(p[i], dtype=np.float32),
            "b": np.ascontiguousarray(b[i], dtype=np.int32),
        }
        for i in range(B)
    ]


def kernel(z, p, b, original_len=None, **_unused) -> np.ndarray:
    z = np.asarray(z, dtype=np.float32)
    p = np.asarray(p, dtype=np.float32)
    b = np.asarray(b, dtype=np.int32)
    assert z.shape == (B, NCH, D) and p.shape == (B, T) and b.shape == (B, T)

    nc = _get_nc()
    res = run_bass_kernel_spmd(nc, make_in_maps(z, p, b), list(range(B)))
    return np.stack([r["out"] for r in res.results], axis=0)
